# revision 15
# baseline (speedup 1.0000x reference)
"""Trainium2 Bass kernel for single-head attention (MDTA-style block).

Reference computation (per batch b, N=4096 tokens, C=128 channels):
    qkv = x @ W_fc + b_fc ; q,k,v = split(qkv)
    S   = (q @ k^T) / sqrt(C)
    A   = softmax(S / scale, axis=-1)
    out = (A @ v) @ W_out + b_out

Sharding: 8 cores = 4 batches x 2 query-halves (data parallel, no
cross-core comm). The token axis is rotated host-side for odd cores so
one SPMD program always sees its queries as tokens 0..2047 (softmax is
invariant to key order).

Per-core algorithm (v4 — flash-style, NxN never hits HBM):
  - q^T, k^T projections in [C, n] layout (bf16 PE matmuls from x^T).
    k-bias dropped (softmax shift-invariance), q-bias kept.
  - scores TRANSPOSED per 128-key tile: S^T[k,q] = kT.T @ qT into a
    [128, 1024] PSUM pair (2 key tiles), exp'd by ONE ScalarE
    activation; the exp stream (1 elem/lane/cycle) is the kernel's
    roofline, so everything else is arranged to keep it saturated.
  - value path via the identity (A@V)@Wo = (A@X) @ (Wv@Wo):
    Z = sum_mt x_mt^T @ E_mt accumulates with RAW x tiles stationary
    (no v projection at all); per 128-query chunk one matmul
    Z_chunk^T @ Wvo lands the result directly in [q, C] row layout.
  - softmax denominator: E pair-tiles accumulate on VectorE into one
    [128,1024] bf16 running sum (one wide add per pair), folded once;
    tiny matmuls esF_chunk^T @ ones give rowsums TRANSPOSED [q, 1] so
    the (8x slow) reciprocal runs on FD=4 only and the normalize is a
    per-partition scalar fused with the bias add.
  - latency hiding: input DMAs split over 4 engine queues (each queue
    streams ~60 GB/s with ~2us latency), projection PSUM lives in the
    tail-only banks so the score-pair double buffer is free from the
    first iteration, 8 zero matmuls warm the PE clock (HAM) during the
    DMA wait, and block tails are emitted two pair-iterations into the
    next block so the PE/ACT streams never drain.
"""

import math
import sys

import numpy as np

sys.path.insert(0, "/opt/trn_rl_repo")

import ml_dtypes  # noqa: E402

import concourse.bacc as bacc  # noqa: E402
import concourse.mybir as mybir  # noqa: E402
import concourse.tile as tile  # noqa: E402
from concourse.bass_utils import run_bass_kernel_spmd  # noqa: E402

B, N, C = 4, 4096, 128
NCORES = 8
NQ = N // 2  # queries per core
NB = 512  # query block size
NMT = N // C  # key tiles (32)
F32 = mybir.dt.float32
BF16 = mybir.dt.bfloat16
AOP = mybir.AluOpType
ACT = mybir.ActivationFunctionType

_cache: dict = {}
LAST_RESULTS = None


def _build(sc: float):
    nc = bacc.Bacc(None, target_bir_lowering=False, debug=True)

    xT = nc.declare_dram_parameter("xT", [C, N], BF16, isOutput=False)
    xN = nc.declare_dram_parameter("xN", [C, NMT * C], BF16, isOutput=False)
    Wp = nc.declare_dram_parameter("Wp", [C, 3 * C], BF16, isOutput=False)
    bp = nc.declare_dram_parameter("bp", [C, C + 1], F32, isOutput=False)
    y = nc.declare_dram_parameter("y", [NQ, C], F32, isOutput=True)

    with tile.TileContext(nc) as tc:
        with (
            tc.tile_pool(name="const", bufs=1) as cp,
            tc.tile_pool(name="ebuf", bufs=3) as ep,
            tc.tile_pool(name="esum", bufs=2) as esp,
            tc.tile_pool(name="small", bufs=2) as sp,
            tc.tile_pool(name="ybuf", bufs=3) as yp,
            tc.tile_pool(name="ps_s", bufs=2, space="PSUM") as ps_s,
            tc.tile_pool(name="ps_z", bufs=2, space="PSUM") as ps_z,
            tc.tile_pool(name="ps_rs", bufs=1, space="PSUM") as ps_rs,
            tc.tile_pool(name="ps_y", bufs=1, space="PSUM") as ps_y,
        ):
            xT_s = cp.tile([C, N], BF16)
            xN_s = cp.tile([C, NMT * C], BF16)
            wp_s = cp.tile([C, 3 * C], BF16)
            bp_s = cp.tile([C, C + 1], F32)
            ones_s = cp.tile([C, 1], BF16)
            warm_s = cp.tile([C, NB], BF16)
            kT_s = cp.tile([C, N], BF16)
            qT_s = cp.tile([C, NQ], BF16)
            wq_s = wp_s[:, 0:C]
            wk_s = wp_s[:, C:2 * C]
            wvo_s = wp_s[:, 2 * C:3 * C]
            b2b_s = bp_s[:, 0:C]
            bq_s = bp_s[:, C:C + 1]

            nc.gpsimd.memset(ones_s[:], 1.0)
            nc.gpsimd.memset(warm_s[:], 0.0)
            # inputs split over the 3 DMA-capable queues (sync/scalar/gpsimd)
            nc.scalar.dma_start(out=wp_s[:], in_=Wp[:])
            nc.scalar.dma_start(out=bp_s[:], in_=bp[:])
            for c in [0, 1, 2, 3, 6]:
                sl = slice(c * NB, (c + 1) * NB)
                nc.sync.dma_start(out=xT_s[:, sl], in_=xT[:, sl])
            for c in [4, 5, 7]:
                sl = slice(c * NB, (c + 1) * NB)
                nc.scalar.dma_start(out=xT_s[:, sl], in_=xT[:, sl])
            for g in range(4):
                sl = slice(g * 1024, (g + 1) * 1024)
                nc.gpsimd.dma_start(out=xN_s[:, sl], in_=xN[:, sl])

            # ~3.5us of dummy matmuls releases the HAM clock throttle while
            # the input DMAs stream in
            for i in range(8):
                wu = ps_s.tile([C, NB], F32, tag="s", name="wu")
                nc.tensor.matmul(
                    wu[:], warm_s[:, 0:C], warm_s[:], start=True, stop=True,
                )

            # projections: one [C, 512] PSUM tile each, alternating between
            # the two tail banks (which stay free until the first block tail)
            tailtag = [("rs", ps_rs), ("yp", ps_y)]
            proj_ctr = [0]

            def proj_half(w_ap, dst, col, dve, bias):
                tag, pool = tailtag[proj_ctr[0] % 2]
                proj_ctr[0] += 1
                ps = pool.tile([C, NB], F32, tag=tag, name="ps_proj")
                sl = slice(col, col + NB)
                nc.tensor.matmul(ps[:], w_ap, xT_s[:, sl], start=True, stop=True)
                if bias is not None:
                    dve.tensor_scalar_add(dst[:, sl], ps[:], bias)
                else:
                    dve.tensor_copy(dst[:, sl], ps[:])

            # block 0 only reads qT[:, 0:512] and consumes kT tiles at the
            # exp cadence — emit the other q-bias halves last so the serial
            # VectorE chain never gates the first block's score pairs
            proj_half(wq_s, qT_s, 0 * NB, nc.vector, bq_s)
            for j in range(8):
                proj_half(wk_s, kT_s, j * NB, nc.vector, None)
            proj_half(wq_s, qT_s, 1 * NB, nc.vector, bq_s)
            proj_half(wq_s, qT_s, 2 * NB, nc.vector, bq_s)
            proj_half(wq_s, qT_s, 3 * NB, nc.vector, bq_s)

            pending_tail = [None]

            def make_tail(nb, es, z_ps, last):
                def emit():
                    # per-128-query-chunk pipeline so the final tail's
                    # cross-engine latency chain is as short as possible
                    zT = sp.tile([C, NB], BF16, tag="zT")
                    if last:  # ScalarE is idle during the final tail
                        nc.scalar.copy(zT[:], z_ps[:])
                    else:
                        nc.vector.tensor_copy(zT[:], z_ps[:])
                    esF = sp.tile([C, NB], BF16, tag="esF")
                    rs = ps_rs.tile([C, 4], F32, tag="rs")
                    rcp = sp.tile([C, 4], F32, tag="rcp")
                    for j in range(4):
                        csl = slice(j * C, (j + 1) * C)
                        nc.vector.tensor_tensor(
                            esF[:, csl], es[:, csl],
                            es[:, NB + j * C:NB + (j + 1) * C], op=AOP.add,
                        )
                        nc.tensor.matmul(
                            rs[:, j:j + 1], esF[:, csl], ones_s[:],
                            start=True, stop=True,
                        )
                        nc.vector.reciprocal(rcp[:, j:j + 1], rs[:, j:j + 1])
                        tag, pool = tailtag[(j + 1) % 2]
                        pp = pool.tile([C, C], F32, tag=tag, name="pp")
                        nc.tensor.matmul(
                            pp[:], zT[:, csl], wvo_s, start=True, stop=True,
                        )
                        yt = yp.tile([C, C], F32, tag="yt")
                        nc.vector.scalar_tensor_tensor(
                            yt[:], pp[:], rcp[:, j:j + 1], b2b_s,
                            op0=AOP.mult, op1=AOP.add,
                        )
                        r0 = nb * NB + j * C
                        eng = nc.sync if j % 2 == 0 else nc.gpsimd
                        eng.dma_start(out=y[r0:r0 + C, :], in_=yt[:])
                return emit

            for nb in range(NQ // NB):
                qsl = slice(nb * NB, (nb + 1) * NB)
                z_ps = ps_z.tile([C, NB], F32, tag="z")
                es = esp.tile([C, 2 * NB], BF16, tag="es")
                E_prev = None
                for mp in range(NMT // 2):
                    asl = slice(2 * mp * C, (2 * mp + 1) * C)
                    bsl = slice((2 * mp + 1) * C, (2 * mp + 2) * C)
                    pss = ps_s.tile([128, 2 * NB], F32, tag="s")
                    nc.tensor.matmul(
                        pss[:, 0:NB], kT_s[:, asl], qT_s[:, qsl],
                        start=True, stop=True,
                    )
                    nc.tensor.matmul(
                        pss[:, NB:2 * NB], kT_s[:, bsl], qT_s[:, qsl],
                        start=True, stop=True,
                    )
                    if mp == 2 and pending_tail[0] is not None:
                        pending_tail[0]()
                        pending_tail[0] = None
                    E = ep.tile([128, 2 * NB], BF16, tag="E")
                    nc.scalar.activation(E[:], pss[:], ACT.Exp, scale=sc)
                    nc.tensor.matmul(
                        z_ps[:], xN_s[:, asl], E[:, 0:NB],
                        start=(mp == 0), stop=False,
                    )
                    nc.tensor.matmul(
                        z_ps[:], xN_s[:, bsl], E[:, NB:2 * NB],
                        start=False, stop=(mp == NMT // 2 - 1),
                    )
                    if mp == 0:
                        E_prev = E
                    elif mp == 1:
                        nc.vector.tensor_tensor(
                            es[:], E_prev[:], E[:], op=AOP.add,
                        )
                        E_prev = None
                    else:
                        nc.vector.tensor_tensor(es[:], es[:], E[:], op=AOP.add)
                pending_tail[0] = make_tail(nb, es, z_ps, nb == NQ // NB - 1)
            pending_tail[0]()

    nc.compile()
    return nc


def kernel(x, W_fc, b_fc, W_out, b_out, scale):
    x = np.asarray(x, dtype=np.float32)
    W_fc = np.asarray(W_fc, dtype=np.float32)
    b_fc = np.asarray(b_fc, dtype=np.float32)
    W_out = np.asarray(W_out, dtype=np.float32)
    b_out = np.asarray(b_out, dtype=np.float32)
    scale = np.asarray(scale, dtype=np.float32)

    sc = float(1.0 / (math.sqrt(C) * float(scale[0])))
    key = ("v4", sc)
    if key not in _cache:
        _cache.clear()
        _cache[key] = _build(sc)
    nc = _cache[key]

    bf16 = ml_dtypes.bfloat16
    b2 = b_fc[2 * C:] @ W_out + b_out  # v-bias folded through the projection
    Wp = np.concatenate(
        [W_fc[:, :C], W_fc[:, C:2 * C], W_fc[:, 2 * C:] @ W_out], axis=1
    ).astype(bf16)
    bpk = np.concatenate(
        [np.tile(b2, (C, 1)), b_fc[:C].reshape(C, 1)], axis=1
    ).astype(np.float32)
    common = {"Wp": np.ascontiguousarray(Wp), "bp": np.ascontiguousarray(bpk)}
    in_maps = []
    for core in range(NCORES):
        b, h = core // 2, core % 2
        # rotate tokens so this core's queries are rows 0..NQ-1 (key order
        # inside the softmax sum is irrelevant)
        xb = np.roll(x[b], -h * NQ, axis=0) if h else x[b]
        xT_b = np.ascontiguousarray(xb.T).astype(bf16)
        # partition-major tiling: xN[p, mt*C + j] = x[mt*128 + p, j]
        xN_b = np.ascontiguousarray(
            xb.reshape(NMT, C, C).transpose(1, 0, 2).reshape(C, NMT * C)
        ).astype(bf16)
        in_maps.append({**common, "xT": xT_b, "xN": xN_b})

    res = run_bass_kernel_spmd(nc, in_maps, list(range(NCORES)))
    global LAST_RESULTS
    LAST_RESULTS = res

    yout = np.empty((B, N, C), dtype=np.float32)
    for core in range(NCORES):
        b, h = core // 2, core % 2
        yout[b, h * NQ:(h + 1) * NQ, :] = res.results[core]["y"]
    return yout


# revision 20
# speedup vs baseline: 1.0497x; 1.0497x over previous
"""Trainium2 Bass kernel for single-head attention (MDTA-style block).

Reference computation (per batch b, N=4096 tokens, C=128 channels):
    qkv = x @ W_fc + b_fc ; q,k,v = split(qkv)
    S   = (q @ k^T) / sqrt(C)
    A   = softmax(S / scale, axis=-1)
    out = (A @ v) @ W_out + b_out

Sharding: 8 cores = 4 batches x 2 query-halves (data parallel, no
cross-core comm). The token axis is rotated host-side for odd cores so
one SPMD program always sees its queries as tokens 0..2047 (softmax is
invariant to key order).

Per-core algorithm (v4 — flash-style, NxN never hits HBM):
  - q^T, k^T projections in [C, n] layout (bf16 PE matmuls from x^T).
    k-bias dropped (softmax shift-invariance), q-bias kept.
  - scores TRANSPOSED per 128-key tile: S^T[k,q] = kT.T @ qT into a
    [128, 1024] PSUM pair (2 key tiles), exp'd by ONE ScalarE
    activation; the exp stream (1 elem/lane/cycle) is the kernel's
    roofline, so everything else is arranged to keep it saturated.
  - value path via the identity (A@V)@Wo = (A@X) @ (Wv@Wo):
    Z = sum_mt x_mt^T @ E_mt accumulates with RAW x tiles stationary
    (no v projection at all); per 128-query chunk one matmul
    Z_chunk^T @ Wvo lands the result directly in [q, C] row layout.
  - softmax denominator: E pair-tiles accumulate on VectorE into one
    [128,1024] bf16 running sum (one wide add per pair), folded once;
    tiny matmuls esF_chunk^T @ ones give rowsums TRANSPOSED [q, 1] so
    the (8x slow) reciprocal runs on FD=4 only and the normalize is a
    per-partition scalar fused with the bias add.
  - latency hiding: input DMAs split over 4 engine queues (each queue
    streams ~60 GB/s with ~2us latency), projection PSUM lives in the
    tail-only banks so the score-pair double buffer is free from the
    first iteration, 8 zero matmuls warm the PE clock (HAM) during the
    DMA wait, and block tails are emitted two pair-iterations into the
    next block so the PE/ACT streams never drain.
"""

import math
import sys

import numpy as np

sys.path.insert(0, "/opt/trn_rl_repo")

import ml_dtypes  # noqa: E402

import concourse.bacc as bacc  # noqa: E402
import concourse.mybir as mybir  # noqa: E402
import concourse.tile as tile  # noqa: E402
from concourse.bass_utils import run_bass_kernel_spmd  # noqa: E402

B, N, C = 4, 4096, 128
NCORES = 8
NQ = N // 2  # queries per core
NB = 512  # query block size
NMT = N // C  # key tiles (32)
F32 = mybir.dt.float32
BF16 = mybir.dt.bfloat16
AOP = mybir.AluOpType
ACT = mybir.ActivationFunctionType

_cache: dict = {}
LAST_RESULTS = None


def _build(sc: float):
    nc = bacc.Bacc(None, target_bir_lowering=False, debug=True)

    xT = nc.declare_dram_parameter("xT", [C, N], BF16, isOutput=False)
    xN = nc.declare_dram_parameter("xN", [C, NMT * C], BF16, isOutput=False)
    Wp = nc.declare_dram_parameter("Wp", [C, 3 * C], BF16, isOutput=False)
    bp = nc.declare_dram_parameter("bp", [C, C + 1], F32, isOutput=False)
    y = nc.declare_dram_parameter("y", [NQ, C], F32, isOutput=True)

    with tile.TileContext(nc) as tc:
        with (
            tc.tile_pool(name="const", bufs=1) as cp,
            tc.tile_pool(name="ebuf", bufs=4) as ep,
            tc.tile_pool(name="esum", bufs=2) as esp,
            tc.tile_pool(name="small", bufs=2) as sp,
            tc.tile_pool(name="ybuf", bufs=5) as yp,
            tc.tile_pool(name="ps_s", bufs=2, space="PSUM") as ps_s,
            tc.tile_pool(name="ps_z", bufs=2, space="PSUM") as ps_z,
            tc.tile_pool(name="ps_rs", bufs=1, space="PSUM") as ps_rs,
            tc.tile_pool(name="ps_y", bufs=1, space="PSUM") as ps_y,
        ):
            xT_s = cp.tile([C, N], BF16)
            xN_s = cp.tile([C, NMT * C], BF16)
            wp_s = cp.tile([C, 3 * C], BF16)
            bp_s = cp.tile([C, C + 1], F32)
            ones_s = cp.tile([C, 1], BF16)
            warm_s = cp.tile([C, NB], BF16)
            kT_s = cp.tile([C, N], BF16)
            qT_s = cp.tile([C, NQ], BF16)
            wq_s = wp_s[:, 0:C]
            wk_s = wp_s[:, C:2 * C]
            wvo_s = wp_s[:, 2 * C:3 * C]
            b2b_s = bp_s[:, 0:C]
            bq_s = bp_s[:, C:C + 1]

            nc.gpsimd.memset(ones_s[:], 1.0)
            nc.gpsimd.memset(warm_s[:], 0.0)
            # inputs split over the 3 DMA-capable queues (sync/scalar/gpsimd)
            nc.scalar.dma_start(out=wp_s[:], in_=Wp[:])
            nc.scalar.dma_start(out=bp_s[:], in_=bp[:])
            for c in [0, 1, 2, 3, 6]:
                sl = slice(c * NB, (c + 1) * NB)
                nc.sync.dma_start(out=xT_s[:, sl], in_=xT[:, sl])
            for c in [4, 5, 7]:
                sl = slice(c * NB, (c + 1) * NB)
                nc.scalar.dma_start(out=xT_s[:, sl], in_=xT[:, sl])
            for g in range(4):
                sl = slice(g * 1024, (g + 1) * 1024)
                nc.gpsimd.dma_start(out=xN_s[:, sl], in_=xN[:, sl])

            # ~3.5us of dummy matmuls releases the HAM clock throttle while
            # the input DMAs stream in
            for i in range(8):
                wu = ps_s.tile([C, NB], F32, tag="s", name="wu")
                nc.tensor.matmul(
                    wu[:], warm_s[:, 0:C], warm_s[:], start=True, stop=True,
                )

            # projections: one [C, 512] PSUM tile each, alternating between
            # the two tail banks (which stay free until the first block tail)
            tailtag = [("rs", ps_rs), ("yp", ps_y)]
            proj_ctr = [0]

            def proj_half(w_ap, dst, col, dve, bias):
                tag, pool = tailtag[proj_ctr[0] % 2]
                proj_ctr[0] += 1
                ps = pool.tile([C, NB], F32, tag=tag, name="ps_proj")
                sl = slice(col, col + NB)
                nc.tensor.matmul(ps[:], w_ap, xT_s[:, sl], start=True, stop=True)
                if bias is not None:
                    dve.tensor_scalar_add(dst[:, sl], ps[:], bias)
                else:
                    dve.tensor_copy(dst[:, sl], ps[:])

            # Engine queues run in order, so a projection matmul that waits
            # on the serial VectorE cast chain would head-of-line block the
            # score stream. Emit upfront only what block 0 needs first
            # (its q half, kT tiles 0-7); the rest drips through block 0's
            # pair loop ~2 iterations before each piece is consumed.
            proj_half(wq_s, qT_s, 0 * NB, nc.vector, bq_s)
            proj_half(wk_s, kT_s, 0 * NB, nc.vector, None)
            proj_half(wk_s, kT_s, 1 * NB, nc.vector, None)
            deferred_proj = [
                lambda: proj_half(wk_s, kT_s, 2 * NB, nc.vector, None),
                lambda: proj_half(wk_s, kT_s, 3 * NB, nc.vector, None),
                lambda: proj_half(wk_s, kT_s, 4 * NB, nc.vector, None),
                lambda: proj_half(wk_s, kT_s, 5 * NB, nc.vector, None),
                lambda: proj_half(wk_s, kT_s, 6 * NB, nc.vector, None),
                lambda: proj_half(wk_s, kT_s, 7 * NB, nc.vector, None),
                lambda: proj_half(wq_s, qT_s, 1 * NB, nc.vector, bq_s),
                lambda: proj_half(wq_s, qT_s, 2 * NB, nc.vector, bq_s),
                lambda: proj_half(wq_s, qT_s, 3 * NB, nc.vector, bq_s),
            ]

            pending_tail = [None]

            def make_tail(nb, es, z_ps, last):
                def emit():
                    # per-128-query-chunk pipeline so the final tail's
                    # cross-engine latency chain is as short as possible
                    zT = sp.tile([C, NB], BF16, tag="zT")
                    if last:  # ScalarE is idle during the final tail
                        nc.scalar.copy(zT[:], z_ps[:])
                    else:
                        nc.vector.tensor_copy(zT[:], z_ps[:])
                    esF = sp.tile([C, NB], BF16, tag="esF")
                    rs = ps_rs.tile([C, 4], F32, tag="rs")
                    rcp = sp.tile([C, 4], F32, tag="rcp")
                    for j in range(4):
                        csl = slice(j * C, (j + 1) * C)
                        nc.vector.tensor_tensor(
                            esF[:, csl], es[:, csl],
                            es[:, NB + j * C:NB + (j + 1) * C], op=AOP.add,
                        )
                        nc.tensor.matmul(
                            rs[:, j:j + 1], esF[:, csl], ones_s[:],
                            start=True, stop=True,
                        )
                        nc.vector.reciprocal(rcp[:, j:j + 1], rs[:, j:j + 1])
                        tag, pool = tailtag[(j + 1) % 2]
                        pp = pool.tile([C, C], F32, tag=tag, name="pp")
                        nc.tensor.matmul(
                            pp[:], zT[:, csl], wvo_s, start=True, stop=True,
                        )
                        yt = yp.tile([C, C], F32, tag="yt")
                        nc.vector.scalar_tensor_tensor(
                            yt[:], pp[:], rcp[:, j:j + 1], b2b_s,
                            op0=AOP.mult, op1=AOP.add,
                        )
                        r0 = nb * NB + j * C
                        if last:  # all 3 DMA queues; ScalarE is idle here
                            eng = [nc.sync, nc.gpsimd, nc.scalar, nc.sync][j]
                        else:
                            eng = nc.sync if j % 2 == 0 else nc.gpsimd
                        eng.dma_start(out=y[r0:r0 + C, :], in_=yt[:])
                return emit

            for nb in range(NQ // NB):
                qsl = slice(nb * NB, (nb + 1) * NB)
                z_ps = ps_z.tile([C, NB], F32, tag="z")
                es = esp.tile([C, 2 * NB], BF16, tag="es")
                E_prev = None
                for mp in range(NMT // 2):
                    asl = slice(2 * mp * C, (2 * mp + 1) * C)
                    bsl = slice((2 * mp + 1) * C, (2 * mp + 2) * C)
                    pss = ps_s.tile([128, 2 * NB], F32, tag="s")
                    nc.tensor.matmul(
                        pss[:, 0:NB], kT_s[:, asl], qT_s[:, qsl],
                        start=True, stop=True,
                    )
                    nc.tensor.matmul(
                        pss[:, NB:2 * NB], kT_s[:, bsl], qT_s[:, qsl],
                        start=True, stop=True,
                    )
                    if nb == 0 and mp < len(deferred_proj):
                        deferred_proj[mp]()
                    if mp == 2 and pending_tail[0] is not None:
                        pending_tail[0]()
                        pending_tail[0] = None
                    E = ep.tile([128, 2 * NB], BF16, tag="E")
                    nc.scalar.activation(E[:], pss[:], ACT.Exp, scale=sc)
                    nc.tensor.matmul(
                        z_ps[:], xN_s[:, asl], E[:, 0:NB],
                        start=(mp == 0), stop=False,
                    )
                    nc.tensor.matmul(
                        z_ps[:], xN_s[:, bsl], E[:, NB:2 * NB],
                        start=False, stop=(mp == NMT // 2 - 1),
                    )
                    if mp == 0:
                        E_prev = E
                    elif mp == 1:
                        nc.vector.tensor_tensor(
                            es[:], E_prev[:], E[:], op=AOP.add,
                        )
                        E_prev = None
                    else:
                        nc.vector.tensor_tensor(es[:], es[:], E[:], op=AOP.add)
                pending_tail[0] = make_tail(nb, es, z_ps, nb == NQ // NB - 1)
            pending_tail[0]()

    nc.compile()
    return nc


def kernel(x, W_fc, b_fc, W_out, b_out, scale):
    x = np.asarray(x, dtype=np.float32)
    W_fc = np.asarray(W_fc, dtype=np.float32)
    b_fc = np.asarray(b_fc, dtype=np.float32)
    W_out = np.asarray(W_out, dtype=np.float32)
    b_out = np.asarray(b_out, dtype=np.float32)
    scale = np.asarray(scale, dtype=np.float32)

    sc = float(1.0 / (math.sqrt(C) * float(scale[0])))
    key = ("v4", sc)
    if key not in _cache:
        _cache.clear()
        _cache[key] = _build(sc)
    nc = _cache[key]

    bf16 = ml_dtypes.bfloat16
    b2 = b_fc[2 * C:] @ W_out + b_out  # v-bias folded through the projection
    Wp = np.concatenate(
        [W_fc[:, :C], W_fc[:, C:2 * C], W_fc[:, 2 * C:] @ W_out], axis=1
    ).astype(bf16)
    bpk = np.concatenate(
        [np.tile(b2, (C, 1)), b_fc[:C].reshape(C, 1)], axis=1
    ).astype(np.float32)
    common = {"Wp": np.ascontiguousarray(Wp), "bp": np.ascontiguousarray(bpk)}
    in_maps = []
    for core in range(NCORES):
        b, h = core // 2, core % 2
        # rotate tokens so this core's queries are rows 0..NQ-1 (key order
        # inside the softmax sum is irrelevant)
        xb = np.roll(x[b], -h * NQ, axis=0) if h else x[b]
        xT_b = np.ascontiguousarray(xb.T).astype(bf16)
        # partition-major tiling: xN[p, mt*C + j] = x[mt*128 + p, j]
        xN_b = np.ascontiguousarray(
            xb.reshape(NMT, C, C).transpose(1, 0, 2).reshape(C, NMT * C)
        ).astype(bf16)
        in_maps.append({**common, "xT": xT_b, "xN": xN_b})

    res = run_bass_kernel_spmd(nc, in_maps, list(range(NCORES)))
    global LAST_RESULTS
    LAST_RESULTS = res

    yout = np.empty((B, N, C), dtype=np.float32)
    for core in range(NCORES):
        b, h = core // 2, core % 2
        yout[b, h * NQ:(h + 1) * NQ, :] = res.results[core]["y"]
    return yout
